# revision 9
# baseline (speedup 1.0000x reference)
"""AdaptiveRoutingLayer kernel for 8 TRN2 NeuronCores.

Math: out = sum_e softmax(routing_weights[task_id])[e] * (x @ W[e].T + b[e])
The weighted sum over experts is linear, so it collapses to a single matmul:
    out = x @ Wmix.T + bmix,  Wmix = sum_e w[e] * W[e],  bmix = sum_e w[e] * b[e]
Host mixes the weights (cheap: E*D*D MACs); the device does the B x D x D
matmul, data-parallel over the 8 cores (1024 tokens each). No collectives.

Device kernel: per 512-col PSUM chain, 12 of 16 k-tiles run in bf16 (216 ns/MM
warm) and the last 4 k-tiles run as 2 fp8e4m3 DoubleRow pair-matmuls (K=256
per MM), cutting the PE stream ~11%. fp8 operands are host-quantized with
balanced scales (s_x * s_w = 1, so no descale pass); measured end-to-end
rel err 1.62e-2 vs the fp32 reference (gate: 2e-2).

Schedule: k-tile DMAs are grouped (one issue per 3-4 k-tiles, small first
tiles) and spread over 4 engine queues so delivery outruns the PE from the
first matmul; tiny PE warmups cover the DMA head so the HAM activity window
opens during it; per-bank (512-col) evictions with a deep buffer ring keep
DVE from ever blocking on out-DMA completion.
"""

import numpy as np
import ml_dtypes

# Problem shapes (hardcoded; kernel.py must be self-contained).
E, T, D, B = 8, 4, 2048, 8192
N_CORES = 8
B_SH = B // N_CORES          # 1024 tokens per core
P = 128                      # SBUF partitions
NK8 = 4                      # k-tiles carried in fp8 (must be even)
NPAIR = NK8 // 2             # fp8 DoubleRow pair-matmuls per chain
KT_BF = D // P - NK8         # 12 bf16 k-tiles
KT_TOT = KT_BF + NPAIR       # 14 PE slots per chain
K_BF = KT_BF * P             # 1536 bf16-contracted K elements
HB = B_SH // 2               # 512-token halves (m groups)
HD = D // 2                  # 1024-col halves of the output / W
NTILE = 512                  # matmul free dim (one PSUM bank of fp32)

# k-tile DMA groups: small head tiles so the first matmuls start ~2us in,
# larger groups after so issue rate never bounds delivery.
XA_GROUPS = [(0, 1), (1, 1), (2, 3), (5, 4), (9, 3)]
WH0_GROUPS = [(0, 1), (1, 1), (2, 3), (5, 4), (9, 3)]
BIG_GROUPS = [(0, 4), (4, 4), (8, 4)]

_CACHE = {}


def _build():
    """Build + compile the per-core Bass/Tile graph (same program on all 8 cores)."""
    import concourse.bacc as bacc
    import concourse.mybir as mybir
    import concourse.tile as tile

    nc = bacc.Bacc("TRN2", target_bir_lowering=False, debug=False,
                   num_devices=N_CORES)

    bf16 = mybir.dt.bfloat16
    f8 = mybir.dt.float8e4
    f32 = mybir.dt.float32

    # DRAM layouts are host-packed [partition, k-tile, free] so one DMA can
    # fetch a contiguous k-tile group into one SBUF tile.
    xbf = nc.dram_tensor("xbf", [P, KT_BF, B_SH], bf16, kind="ExternalInput").ap()
    wbf = nc.dram_tensor("wbf", [P, KT_BF, D], bf16, kind="ExternalInput").ap()
    bias = nc.dram_tensor("bias", [P, D], bf16, kind="ExternalInput").ap()
    out = nc.dram_tensor("out", [B_SH, D], bf16, kind="ExternalOutput").ap()
    x8d = {}
    w8d = {}
    for a in range(NPAIR):
        for g in ("a", "b"):
            x8d[(a, g)] = nc.dram_tensor(
                f"x8{g}{a}", [P, 2, HB], f8, kind="ExternalInput").ap()
        for h in range(2):
            w8d[(a, h)] = nc.dram_tensor(
                f"w8_{a}{h}", [P, 2, HD], f8, kind="ExternalInput").ap()

    with tile.TileContext(nc) as tc:
        with (
            tc.tile_pool(name="wpool", bufs=1) as wpool,
            tc.tile_pool(name="xpool", bufs=1) as xpool,
            tc.tile_pool(name="bpool", bufs=1) as bpool,
            tc.tile_pool(name="opool", bufs=10) as opool,
            tc.tile_pool(name="pspool", bufs=1, space="PSUM") as pspool,
        ):
            # Whole working set is SBUF-resident (~12.5 MiB). Group tiles
            # hold several k-tiles; (tile, local index) per logical k-tile.
            def make_groups(pool, groups, width, dt, pfx):
                tiles = {}
                for g0, n in groups:
                    t = pool.tile([P, n, width], dt, name=f"{pfx}{g0}",
                                  tag=f"{pfx}{g0}")
                    for j in range(n):
                        tiles[g0 + j] = (t, j)
                return tiles

            xa_t = make_groups(xpool, XA_GROUPS, HB, bf16, "xa")
            xb_t = make_groups(xpool, BIG_GROUPS, HB, bf16, "xb")
            w0_t = make_groups(wpool, WH0_GROUPS, HD, bf16, "w0_")
            w1_t = make_groups(wpool, BIG_GROUPS, HD, bf16, "w1_")
            x8_tiles = {}
            w8_tiles = {}
            for a in range(NPAIR):
                for g in ("a", "b"):
                    x8_tiles[(a, g)] = xpool.tile(
                        [P, 2, HB], f8, name=f"x8{g}{a}", tag=f"x8{g}{a}")
                for h in range(2):
                    w8_tiles[(a, h)] = wpool.tile(
                        [P, 2, HD], f8, name=f"w8_{a}{h}", tag=f"w8_{a}{h}")
            b_s = bpool.tile([P, D], bf16)

            # DMA issue spread over 4 engine queues; within each queue,
            # issue order == PE consumption order.
            def grp_dma(eng, tiles, groups, src, width, w0=0):
                for g0, n in groups:
                    t, _ = tiles[g0]
                    eng.dma_start(t[:], src[:, g0:g0 + n, w0:w0 + width])

            # Warm tile init on the (otherwise idle) DVE so PE warmups are
            # never queued behind DMA issues.
            warm = bpool.tile([P, P], bf16, name="warm")
            nc.vector.memset(warm[:], 0.0)

            # Only Sync and Scalar have hardware descriptor generation
            # (GpSimd DMA is SWDGE: microseconds of startup + completion
            # latency), so all latency-sensitive DMAs go on these two.
            # Sync: x stream, then output evictions (emitted in the pass
            # loop below). Scalar: w stream + bias + fp8 tiles.
            grp_dma(nc.sync, xa_t, XA_GROUPS, xbf, HB)
            grp_dma(nc.sync, xb_t, BIG_GROUPS, xbf, HB, w0=HB)
            for a in range(NPAIR):
                nc.sync.dma_start(x8_tiles[(a, "b")][:], x8d[(a, "b")][:])
            t00, _ = w0_t[0]
            nc.scalar.dma_start(t00[:, 0, 0:NTILE], wbf[:, 0, 0:NTILE])
            nc.scalar.dma_start(t00[:, 0, NTILE:HD], wbf[:, 0, NTILE:HD])
            grp_dma(nc.scalar, w0_t, WH0_GROUPS[1:], wbf, HD)
            for a in range(NPAIR):
                nc.scalar.dma_start(w8_tiles[(a, 0)][:], w8d[(a, 0)][:])
            nc.scalar.dma_start(b_s[:], bias[:])  # needed at first eviction (~28us)
            for a in range(NPAIR):
                nc.scalar.dma_start(x8_tiles[(a, "a")][:], x8d[(a, "a")][:])
            grp_dma(nc.scalar, w1_t, BIG_GROUPS, wbf, HD, w0=HD)
            for a in range(NPAIR):
                nc.scalar.dma_start(w8_tiles[(a, 1)][:], w8d[(a, 1)][:])

            # PE warm-up: small dummy matmuls with no DMA deps cover the
            # initial DMA-head (~2us) so the HAM activity window starts
            # accumulating immediately; real matmuls follow as soon as the
            # first tiles land and ride out the rest of the cold window on
            # real work.
            first = True
            for mg, h in ((0, 0), (1, 0), (0, 1), (1, 1)):
                ps = [pspool.tile([P, HD], f32, name=f"ps{mg}{h}{i}", tag=f"ps{i}")
                      for i in range(4)]
                if first:
                    first = False
                    # Warm-up bridge: ~4.5us of dummy matmuls keeps the PE
                    # busy from ~1us in, so the HAM activity window flips to
                    # 2.4 GHz during the bridge and the real stream starts
                    # warm, just as the first k-tiles' DMA completions land
                    # (HWDGE completion latency is ~2-4us). Cold real
                    # matmuls and HAM resets from head stalls both vanish.
                    for _ in range(48):
                        nc.tensor.matmul(ps[0][:, 0:P], warm[:], warm[:],
                                         start=True, stop=True)
                last_pass = (mg, h) == (1, 1)
                # Stagger the 4 accumulation chains so they stop at different
                # points: evictions and out-DMAs pipeline against the
                # remaining matmuls instead of bursting at the pass boundary,
                # and the next pass's start-matmuls never wait on them. Pass
                # 1 uses a shallow stagger (its head is DMA-delivery-bound);
                # the last pass's deep stagger leaves only chain 3's eviction
                # in the kernel tail.
                delta = (0, 1, 2, 3) if (mg, h) == (0, 0) else (0, 4, 8, 12)
                sched = [(i, v - delta[i])
                         for v in range(KT_TOT + delta[-1]) for i in range(4)
                         if 0 <= v - delta[i] < KT_TOT]
                xh = xa_t if mg == 0 else xb_t
                wh = w0_t if h == 0 else w1_t
                g = "a" if mg == 0 else "b"
                for i, kt in sched:
                    if kt < KT_BF:
                        xt, xj = xh[kt]
                        wt, wj = wh[kt]
                        lhsT = xt[:, xj, i * P:(i + 1) * P]       # [K=128, M=128]
                        for n2 in range(2):
                            nc.tensor.matmul(
                                ps[i][:, n2 * NTILE:(n2 + 1) * NTILE],
                                lhsT,
                                wt[:, wj, n2 * NTILE:(n2 + 1) * NTILE],
                                start=(kt == 0),
                                stop=False,
                            )
                    else:
                        a = kt - KT_BF
                        lhsT = x8_tiles[(a, g)][:, :, i * P:(i + 1) * P]  # [128,2,128]
                        for n2 in range(2):
                            nc.tensor.matmul(
                                ps[i][:, n2 * NTILE:(n2 + 1) * NTILE],
                                lhsT,
                                w8_tiles[(a, h)][:, :, n2 * NTILE:(n2 + 1) * NTILE],
                                start=False,
                                stop=(kt == KT_TOT - 1),
                                perf_mode=mybir.MatmulPerfMode.DoubleRow,
                            )
                # Per-bank (512-col) evictions pipeline DVE + out-DMA against
                # the next pass's matmuls.
                for i in range(4):
                    m = mg * 4 + i
                    for n2 in range(2):
                        sl = slice(n2 * NTILE, (n2 + 1) * NTILE)
                        gl = slice(h * HD + n2 * NTILE, h * HD + (n2 + 1) * NTILE)
                        o_t = opool.tile([P, NTILE], bf16,
                                         name=f"o{mg}{h}{i}{n2}", tag="o")
                        nc.vector.tensor_add(o_t[:], ps[i][:, sl], b_s[:, gl])
                        nc.sync.dma_start(out[m * P:(m + 1) * P, gl], o_t[:])

    nc.compile()
    return nc


def _mix(W, b, routing_weights, task_id):
    tid = int(np.asarray(task_id))
    r = np.asarray(routing_weights, np.float64)[tid]
    w = np.exp(r - r.max())
    w = (w / w.sum()).astype(np.float32)                 # [E]
    Wmix = np.tensordot(w, np.asarray(W, np.float32), axes=([0], [0]))  # [Do, Di]
    bmix = (w[:, None] * np.asarray(b, np.float32)).sum(0)              # [D]
    return Wmix, bmix


def _make_in_maps(x, W, b, routing_weights, task_id):
    f8 = ml_dtypes.float8_e4m3
    Wmix, bmix = _mix(W, b, routing_weights, task_id)
    WmixT = np.ascontiguousarray(Wmix.T)                                # [Di, Do]
    bias = np.ascontiguousarray(
        np.broadcast_to(bmix, (P, D))).astype(ml_dtypes.bfloat16)
    xT = np.asarray(x, np.float32).T                                    # [D, B]

    # [p, kt, free] packing so grouped k-tile DMAs are contiguous slices
    xbf_full = np.ascontiguousarray(
        xT[:K_BF].reshape(KT_BF, P, B).transpose(1, 0, 2)
    ).astype(ml_dtypes.bfloat16)                                        # [P,12,B]
    wbf = np.ascontiguousarray(
        WmixT[:K_BF].reshape(KT_BF, P, D).transpose(1, 0, 2)
    ).astype(ml_dtypes.bfloat16)                                        # [P,12,D]

    # fp8 slice with balanced scales: s1*s2 == 1 so no descale is needed on
    # device; the geometric split keeps both operands clear of the e4m3
    # denormal floor.
    s1 = np.float32(np.sqrt(Wmix.std()))
    s2 = np.float32(1.0) / s1
    x8_full = np.clip(xT[K_BF:] * s1, -240, 240).astype(f8)             # [512, B]
    w8_full = np.clip(WmixT[K_BF:] * s2, -240, 240).astype(f8)          # [512, D]
    w8r = w8_full.reshape(NPAIR, 2, P, D)                               # [a,s,p,n]

    common = {"wbf": wbf, "bias": bias}
    for a in range(NPAIR):
        for h in range(2):
            common[f"w8_{a}{h}"] = np.ascontiguousarray(
                w8r[a, :, :, h * HD:(h + 1) * HD].transpose(1, 0, 2))   # [p,s,n]

    in_maps = []
    for c in range(N_CORES):
        m = dict(common)
        m["xbf"] = np.ascontiguousarray(xbf_full[:, :, c * B_SH:(c + 1) * B_SH])
        x8c = x8_full[:, c * B_SH:(c + 1) * B_SH].reshape(NPAIR, 2, P, B_SH)
        for a in range(NPAIR):
            for g, t0 in (("a", 0), ("b", HB)):
                m[f"x8{g}{a}"] = np.ascontiguousarray(
                    x8c[a, :, :, t0:t0 + HB].transpose(1, 0, 2))        # [p,s,t]
        in_maps.append(m)
    return in_maps


def kernel(x, W, b, routing_weights, task_id):
    from concourse.bass_utils import run_bass_kernel_spmd

    in_maps = _make_in_maps(x, W, b, routing_weights, task_id)
    if "nc" not in _CACHE:
        _CACHE["nc"] = _build()
    nc = _CACHE["nc"]
    res = run_bass_kernel_spmd(nc, in_maps, core_ids=list(range(N_CORES)))
    return np.concatenate([res.results[c]["out"] for c in range(N_CORES)],
                          axis=0).astype(np.float32)


# revision 12
# speedup vs baseline: 1.0156x; 1.0156x over previous
"""AdaptiveRoutingLayer kernel for 8 TRN2 NeuronCores.

Math: out = sum_e softmax(routing_weights[task_id])[e] * (x @ W[e].T + b[e])
The weighted sum over experts is linear, so it collapses to a single matmul:
    out = x @ Wmix.T + bmix,  Wmix = sum_e w[e] * W[e],  bmix = sum_e w[e] * b[e]
Host mixes the weights (cheap: E*D*D MACs); the device does the B x D x D
matmul, data-parallel over the 8 cores (1024 tokens each). No collectives.

Device kernel: per 512-col PSUM chain, 12 of 16 k-tiles run in bf16 (216 ns/MM
warm) and the last 4 k-tiles run as 2 fp8e4m3 DoubleRow pair-matmuls (K=256
per MM), cutting the PE stream ~11%. fp8 operands are host-quantized with
balanced scales (s_x * s_w = 1, so no descale pass); measured end-to-end
rel err 1.62e-2 vs the fp32 reference (gate: 2e-2).

Schedule: k-tile DMAs are grouped (one issue per 3-4 k-tiles, small first
tiles) and spread over 4 engine queues so delivery outruns the PE from the
first matmul; tiny PE warmups cover the DMA head so the HAM activity window
opens during it; per-bank (512-col) evictions with a deep buffer ring keep
DVE from ever blocking on out-DMA completion.
"""

import numpy as np
import ml_dtypes

# Problem shapes (hardcoded; kernel.py must be self-contained).
E, T, D, B = 8, 4, 2048, 8192
N_CORES = 8
B_SH = B // N_CORES          # 1024 tokens per core
P = 128                      # SBUF partitions
NK8 = 4                      # k-tiles carried in fp8 (must be even)
NPAIR = NK8 // 2             # fp8 DoubleRow pair-matmuls per chain
KT_BF = D // P - NK8         # 12 bf16 k-tiles
KT_TOT = KT_BF + NPAIR       # 14 PE slots per chain
K_BF = KT_BF * P             # 1536 bf16-contracted K elements
HB = B_SH // 2               # 512-token halves (m groups)
HD = D // 2                  # 1024-col halves of the output / W
NTILE = 512                  # matmul free dim (one PSUM bank of fp32)

# k-tile DMA groups of 4: each HWDGE DMA instruction has a ~2-4us fixed
# descriptor/doorbell/completion latency regardless of size, so fewer,
# larger DMAs deliver k-tiles strictly faster than many small ones.
XA_GROUPS = [(0, 4), (4, 4), (8, 4)]
WH0_GROUPS = [(0, 4), (4, 4), (8, 4)]
BIG_GROUPS = [(0, 4), (4, 4), (8, 4)]

_CACHE = {}


def _build():
    """Build + compile the per-core Bass/Tile graph (same program on all 8 cores)."""
    import concourse.bacc as bacc
    import concourse.mybir as mybir
    import concourse.tile as tile

    nc = bacc.Bacc("TRN2", target_bir_lowering=False, debug=False,
                   num_devices=N_CORES)

    bf16 = mybir.dt.bfloat16
    f8 = mybir.dt.float8e4
    f32 = mybir.dt.float32

    # DRAM layouts are host-packed [partition, k-tile, free] so one DMA can
    # fetch a contiguous k-tile group into one SBUF tile.
    xbf = nc.dram_tensor("xbf", [P, KT_BF, B_SH], bf16, kind="ExternalInput").ap()
    wbf = nc.dram_tensor("wbf", [P, KT_BF, D], bf16, kind="ExternalInput").ap()
    bias = nc.dram_tensor("bias", [P, D], bf16, kind="ExternalInput").ap()
    out = nc.dram_tensor("out", [B_SH, D], bf16, kind="ExternalOutput").ap()
    x8d = {}
    w8d = {}
    for a in range(NPAIR):
        for g in ("a", "b"):
            x8d[(a, g)] = nc.dram_tensor(
                f"x8{g}{a}", [P, 2, HB], f8, kind="ExternalInput").ap()
        for h in range(2):
            w8d[(a, h)] = nc.dram_tensor(
                f"w8_{a}{h}", [P, 2, HD], f8, kind="ExternalInput").ap()

    with tile.TileContext(nc) as tc:
        with (
            tc.tile_pool(name="wpool", bufs=1) as wpool,
            tc.tile_pool(name="xpool", bufs=1) as xpool,
            tc.tile_pool(name="bpool", bufs=1) as bpool,
            tc.tile_pool(name="opool", bufs=10) as opool,
            tc.tile_pool(name="pspool", bufs=1, space="PSUM") as pspool,
        ):
            # Whole working set is SBUF-resident (~12.5 MiB). Group tiles
            # hold several k-tiles; (tile, local index) per logical k-tile.
            def make_groups(pool, groups, width, dt, pfx):
                tiles = {}
                for g0, n in groups:
                    t = pool.tile([P, n, width], dt, name=f"{pfx}{g0}",
                                  tag=f"{pfx}{g0}")
                    for j in range(n):
                        tiles[g0 + j] = (t, j)
                return tiles

            xa_t = make_groups(xpool, XA_GROUPS, HB, bf16, "xa")
            xb_t = make_groups(xpool, BIG_GROUPS, HB, bf16, "xb")
            w0_t = make_groups(wpool, WH0_GROUPS, HD, bf16, "w0_")
            w1_t = make_groups(wpool, BIG_GROUPS, HD, bf16, "w1_")
            x8_tiles = {}
            w8_tiles = {}
            for a in range(NPAIR):
                for g in ("a", "b"):
                    x8_tiles[(a, g)] = xpool.tile(
                        [P, 2, HB], f8, name=f"x8{g}{a}", tag=f"x8{g}{a}")
                for h in range(2):
                    w8_tiles[(a, h)] = wpool.tile(
                        [P, 2, HD], f8, name=f"w8_{a}{h}", tag=f"w8_{a}{h}")
            b_s = bpool.tile([P, D], bf16)

            # DMA issue spread over 4 engine queues; within each queue,
            # issue order == PE consumption order.
            def grp_dma(eng, tiles, groups, src, width, w0=0):
                for g0, n in groups:
                    t, _ = tiles[g0]
                    eng.dma_start(t[:], src[:, g0:g0 + n, w0:w0 + width])

            # Warm tile init on the (otherwise idle) DVE so PE warmups are
            # never queued behind DMA issues.
            warm = bpool.tile([P, P], bf16, name="warm")
            nc.vector.memset(warm[:], 0.0)

            # Only Sync and Scalar have hardware descriptor generation
            # (GpSimd DMA is SWDGE: microseconds of startup + completion
            # latency), so all latency-sensitive DMAs go on these two.
            # Sync: x stream, then output evictions (emitted in the pass
            # loop below). Scalar: w stream + bias + fp8 tiles.
            grp_dma(nc.sync, xa_t, XA_GROUPS, xbf, HB)
            grp_dma(nc.sync, xb_t, BIG_GROUPS, xbf, HB, w0=HB)
            for a in range(NPAIR):
                nc.sync.dma_start(x8_tiles[(a, "b")][:], x8d[(a, "b")][:])
            grp_dma(nc.scalar, w0_t, WH0_GROUPS, wbf, HD)
            for a in range(NPAIR):
                nc.scalar.dma_start(w8_tiles[(a, 0)][:], w8d[(a, 0)][:])
            nc.scalar.dma_start(b_s[:], bias[:])  # needed at first eviction (~28us)
            for a in range(NPAIR):
                nc.scalar.dma_start(x8_tiles[(a, "a")][:], x8d[(a, "a")][:])
            grp_dma(nc.scalar, w1_t, BIG_GROUPS, wbf, HD, w0=HD)
            for a in range(NPAIR):
                nc.scalar.dma_start(w8_tiles[(a, 1)][:], w8d[(a, 1)][:])

            # PE warm-up: small dummy matmuls with no DMA deps cover the
            # initial DMA-head (~2us) so the HAM activity window starts
            # accumulating immediately; real matmuls follow as soon as the
            # first tiles land and ride out the rest of the cold window on
            # real work.
            first = True
            for mg, h in ((0, 0), (1, 0), (0, 1), (1, 1)):
                ps = [pspool.tile([P, HD], f32, name=f"ps{mg}{h}{i}", tag=f"ps{i}")
                      for i in range(4)]
                if first:
                    first = False
                    # Warm-up bridge: ~4.5us of dummy matmuls keeps the PE
                    # busy from ~1us in, so the HAM activity window flips to
                    # 2.4 GHz during the bridge and the real stream starts
                    # warm, just as the first k-tiles' DMA completions land
                    # (HWDGE completion latency is ~2-4us). Cold real
                    # matmuls and HAM resets from head stalls both vanish.
                    for _ in range(56):
                        nc.tensor.matmul(ps[0][:, 0:P], warm[:], warm[:],
                                         start=True, stop=True)
                last_pass = (mg, h) == (1, 1)
                # Stagger the 4 accumulation chains so they stop at different
                # points: evictions and out-DMAs pipeline against the
                # remaining matmuls instead of bursting at the pass boundary,
                # and the next pass's start-matmuls never wait on them. Pass
                # 1 uses a shallow stagger (its head is DMA-delivery-bound);
                # the last pass's deep stagger leaves only chain 3's eviction
                # in the kernel tail.
                delta = (0, 1, 2, 3) if (mg, h) == (0, 0) else (0, 4, 8, 12)
                sched = [(i, v - delta[i])
                         for v in range(KT_TOT + delta[-1]) for i in range(4)
                         if 0 <= v - delta[i] < KT_TOT]
                xh = xa_t if mg == 0 else xb_t
                wh = w0_t if h == 0 else w1_t
                g = "a" if mg == 0 else "b"
                for i, kt in sched:
                    if kt < KT_BF:
                        xt, xj = xh[kt]
                        wt, wj = wh[kt]
                        lhsT = xt[:, xj, i * P:(i + 1) * P]       # [K=128, M=128]
                        for n2 in range(2):
                            nc.tensor.matmul(
                                ps[i][:, n2 * NTILE:(n2 + 1) * NTILE],
                                lhsT,
                                wt[:, wj, n2 * NTILE:(n2 + 1) * NTILE],
                                start=(kt == 0),
                                stop=False,
                            )
                    else:
                        a = kt - KT_BF
                        lhsT = x8_tiles[(a, g)][:, :, i * P:(i + 1) * P]  # [128,2,128]
                        for n2 in range(2):
                            nc.tensor.matmul(
                                ps[i][:, n2 * NTILE:(n2 + 1) * NTILE],
                                lhsT,
                                w8_tiles[(a, h)][:, :, n2 * NTILE:(n2 + 1) * NTILE],
                                start=False,
                                stop=(kt == KT_TOT - 1),
                                perf_mode=mybir.MatmulPerfMode.DoubleRow,
                            )
                # Per-bank (512-col) evictions pipeline DVE + out-DMA against
                # the next pass's matmuls.
                for i in range(4):
                    m = mg * 4 + i
                    for n2 in range(2):
                        sl = slice(n2 * NTILE, (n2 + 1) * NTILE)
                        gl = slice(h * HD + n2 * NTILE, h * HD + (n2 + 1) * NTILE)
                        o_t = opool.tile([P, NTILE], bf16,
                                         name=f"o{mg}{h}{i}{n2}", tag="o")
                        nc.vector.tensor_add(o_t[:], ps[i][:, sl], b_s[:, gl])
                        nc.sync.dma_start(out[m * P:(m + 1) * P, gl], o_t[:])

    nc.compile()
    return nc


def _mix(W, b, routing_weights, task_id):
    tid = int(np.asarray(task_id))
    r = np.asarray(routing_weights, np.float64)[tid]
    w = np.exp(r - r.max())
    w = (w / w.sum()).astype(np.float32)                 # [E]
    Wmix = np.tensordot(w, np.asarray(W, np.float32), axes=([0], [0]))  # [Do, Di]
    bmix = (w[:, None] * np.asarray(b, np.float32)).sum(0)              # [D]
    return Wmix, bmix


def _make_in_maps(x, W, b, routing_weights, task_id):
    f8 = ml_dtypes.float8_e4m3
    Wmix, bmix = _mix(W, b, routing_weights, task_id)
    WmixT = np.ascontiguousarray(Wmix.T)                                # [Di, Do]
    bias = np.ascontiguousarray(
        np.broadcast_to(bmix, (P, D))).astype(ml_dtypes.bfloat16)
    xT = np.asarray(x, np.float32).T                                    # [D, B]

    # [p, kt, free] packing so grouped k-tile DMAs are contiguous slices
    xbf_full = np.ascontiguousarray(
        xT[:K_BF].reshape(KT_BF, P, B).transpose(1, 0, 2)
    ).astype(ml_dtypes.bfloat16)                                        # [P,12,B]
    wbf = np.ascontiguousarray(
        WmixT[:K_BF].reshape(KT_BF, P, D).transpose(1, 0, 2)
    ).astype(ml_dtypes.bfloat16)                                        # [P,12,D]

    # fp8 slice with balanced scales: s1*s2 == 1 so no descale is needed on
    # device; the geometric split keeps both operands clear of the e4m3
    # denormal floor.
    s1 = np.float32(np.sqrt(Wmix.std()))
    s2 = np.float32(1.0) / s1
    x8_full = np.clip(xT[K_BF:] * s1, -240, 240).astype(f8)             # [512, B]
    w8_full = np.clip(WmixT[K_BF:] * s2, -240, 240).astype(f8)          # [512, D]
    w8r = w8_full.reshape(NPAIR, 2, P, D)                               # [a,s,p,n]

    common = {"wbf": wbf, "bias": bias}
    for a in range(NPAIR):
        for h in range(2):
            common[f"w8_{a}{h}"] = np.ascontiguousarray(
                w8r[a, :, :, h * HD:(h + 1) * HD].transpose(1, 0, 2))   # [p,s,n]

    in_maps = []
    for c in range(N_CORES):
        m = dict(common)
        m["xbf"] = np.ascontiguousarray(xbf_full[:, :, c * B_SH:(c + 1) * B_SH])
        x8c = x8_full[:, c * B_SH:(c + 1) * B_SH].reshape(NPAIR, 2, P, B_SH)
        for a in range(NPAIR):
            for g, t0 in (("a", 0), ("b", HB)):
                m[f"x8{g}{a}"] = np.ascontiguousarray(
                    x8c[a, :, :, t0:t0 + HB].transpose(1, 0, 2))        # [p,s,t]
        in_maps.append(m)
    return in_maps


def kernel(x, W, b, routing_weights, task_id):
    from concourse.bass_utils import run_bass_kernel_spmd

    in_maps = _make_in_maps(x, W, b, routing_weights, task_id)
    if "nc" not in _CACHE:
        _CACHE["nc"] = _build()
    nc = _CACHE["nc"]
    res = run_bass_kernel_spmd(nc, in_maps, core_ids=list(range(N_CORES)))
    return np.concatenate([res.results[c]["out"] for c in range(N_CORES)],
                          axis=0).astype(np.float32)
